# revision 75
# baseline (speedup 1.0000x reference)
"""Trainium2 Bass kernel v3 for the 16-head attention block.

Computation (per reference):
    q = y2_for @ Wq + bq ; k = y2_back @ Wk + bk ; v = (y2_for+y2_back) @ Wv + bv
    attn = softmax(q k^T / sqrt(d)) ; out = (attn @ v + y2_for + y2_back) @ Wo + bo

Sharding: 8 cores = 2 batches x 4 head-groups (4 heads / 256 dims each); host
sums the 4 partial output projections per batch and adds bv@Wo + bo.

v3 changes vs v2 (sim body 153us -> 139us):
  - Unified 4-slot PSUM rotation: every PSUM tile is a 2-bank [128,2,512]
    slot in one pool, so QK score tiles can run up to 4 ahead of the exp
    engines instead of 2 (the old st double-buffer starved ScalarE).
  - Chunk-granular software pipeline: the previous combo's PV matmuls, norm
    and out-projection are threaded between the current combo's QK chunks;
    K-projection blocks are produced just-in-time inside combo(0,0).
  - V bias folded to host (softmax rows sum to 1 -> +bv passes through
    attention; host adds bv@Wo), so the V quantize is a plain copy, split
    ACT/DVE.  Residual add moved to GpSimd.  outT is bf16 (halves the
    output DMA; host accumulates in f32).
  - exp() split ScalarE (table exp) / VectorE (Schraudolph bit-trick) with
    per-combo-kind assignment sets tuned against TimelineSim.
  - QK^T stays bf16 with 2-head row tiling; fp8-DoubleRow QK was evaluated
    and rejected (per-j-tile LDWEIGHTS for 4 row-groups exceeds the fill
    saving; PSUM capacity forbids amortizing the stationary across i).
"""

import numpy as np

B, N, DIM, HEADS = 2, 2048, 1024, 16
DH = DIM // HEADS  # 64
P = 128
NG = 4  # head-group shards (cores per batch)
CW = DIM // NG  # 256 columns per core
KO = DIM // P  # 8 contraction tiles
NB = N // P  # 16 j tiles
IB = N // 512  # 4 i-blocks of 512

SHIFT = 4.0  # score shift: probs = exp(s/8 - SHIFT); cancels in normalization
LOG2E = 1.4426950408889634
A8 = 8 * LOG2E / 8.0  # schraudolph slope (scale 1/8 folded in)
B8 = 8 * (7 - 0.0573) - 8 * LOG2E * SHIFT  # 9.3755
# jt indices (0..15) whose exp runs on DVE (rest on ScalarE), per combo kind.
# Prologue combo (0,0): DVE also does projections/V quantize copies.
DVE_JT = (2, 6, 10, 13)  # pair-1 combos (DVE also copies outproj)
DVE_JT_LAST = (2, 6, 10, 13, 15)  # last combo: even engine finish for PV
DVE_JT_P0 = (1, 3, 5, 7, 10, 13)  # pair-0 combos
DVE_JT_PROLOGUE = (1, 3, 6, 9, 11, 14)

_cache = {}


def _build(repeat=1, hw_loop=None):
    from contextlib import ExitStack

    import concourse.mybir as mybir
    import concourse.tile as tile
    from concourse import bacc

    f32 = mybir.dt.float32
    bf16 = mybir.dt.bfloat16
    fp8 = mybir.dt.float8e4

    nc = bacc.Bacc("TRN2", target_bir_lowering=False, debug=False,
                   enable_asserts=False)

    at8 = nc.dram_tensor("at8", [DIM, N], fp8, kind="ExternalInput")
    bt8 = nc.dram_tensor("bt8", [DIM, N], fp8, kind="ExternalInput")
    ct8 = nc.dram_tensor("ct8", [DIM, N], fp8, kind="ExternalInput")
    ctr = nc.dram_tensor("ctr", [CW, N], bf16, kind="ExternalInput")
    wq8 = nc.dram_tensor("wq8", [DIM, CW], fp8, kind="ExternalInput")
    wk8 = nc.dram_tensor("wk8", [DIM, CW], fp8, kind="ExternalInput")
    wv8 = nc.dram_tensor("wv8", [DIM, CW], fp8, kind="ExternalInput")
    wo = nc.dram_tensor("wo", [CW, DIM], bf16, kind="ExternalInput")
    bq = nc.dram_tensor("bq", [CW], f32, kind="ExternalInput")
    bk = nc.dram_tensor("bk", [CW], f32, kind="ExternalInput")
    outT = nc.dram_tensor("outT", [DIM, N], bf16, kind="ExternalOutput")

    with tile.TileContext(nc) as tc, ExitStack() as ctx:
        pools, tiles = _emit_prologue(nc, tc, ctx, mybir, at8, bt8, ct8, ctr,
                                      wq8, wk8, wv8, wo, bq, bk)
        if hw_loop:
            with tc.For_i(0, hw_loop, 1):
                _emit_body(nc, tc, mybir, pools, tiles, outT)
        else:
            for _ in range(repeat):
                _emit_body(nc, tc, mybir, pools, tiles, outT)
    nc.compile()
    return nc


def _emit_prologue(nc, tc, ctx, mybir, at8, bt8, ct8, ctr, wq8, wk8, wv8, wo,
                   bq, bk):
    f32 = mybir.dt.float32
    bf16 = mybir.dt.bfloat16
    fp8 = mybir.dt.float8e4
    AF = mybir.ActivationFunctionType

    const = ctx.enter_context(tc.tile_pool(name="const", bufs=1))
    stage = ctx.enter_context(tc.tile_pool(name="stage", bufs=6))
    rin = ctx.enter_context(tc.tile_pool(name="rin", bufs=2))
    ppool = ctx.enter_context(tc.tile_pool(name="ppool", bufs=8))
    psum = ctx.enter_context(tc.tile_pool(name="psum", bufs=4, space="PSUM"))
    pools = dict(const=const, stage=stage, rin=rin, ppool=ppool, psum=psum)

    t = {}
    t["wq"] = const.tile([P, KO, CW], fp8, tag="wq", name="wq")
    t["wk"] = const.tile([P, KO, CW], fp8, tag="wk", name="wk")
    t["wv"] = const.tile([P, KO, CW], fp8, tag="wv", name="wv")
    t["wo"] = const.tile([P, 2, DIM], bf16, tag="wo", name="wo")
    t["bq"] = const.tile([P, 2], f32, tag="bq", name="bq")
    t["bk"] = const.tile([P, 2], f32, tag="bk", name="bk")
    t["qt"] = const.tile([P, 2, N], bf16, tag="qt", name="qt")
    t["kt"] = const.tile([P, 2, N], bf16, tag="kt", name="kt")
    t["x"] = const.tile([P, 2, N], bf16, tag="x", name="x")
    t["ctr"] = const.tile([P, 2, N], bf16, tag="ctr", name="ctr")
    # V with ones-column at 64; padded to 68 cols for DoubleRow stride rules
    t["v8"] = const.tile([P, KO, 2, 4, DH + 4], fp8, tag="v8", name="v8")
    t["at"] = const.tile([P, KO, N], fp8, tag="at", name="at")
    t["bt"] = const.tile([P, KO, N], fp8, tag="bt", name="bt")
    t["ct"] = const.tile([P, KO, N], fp8, tag="ct", name="ct")
    t["ebias"] = const.tile([P, 1], f32, tag="ebias", name="ebias")
    t["ones1"] = const.tile([1, DH], f32, tag="ones1", name="ones1")
    nc.vector.memset(t["ones1"][:], 1.0)

    # preload exp activation table
    warm = const.tile([1, 8], f32, tag="warm")
    nc.vector.memset(warm[:], 0.0)
    warm2 = const.tile([1, 8], f32, tag="warm2")
    nc.scalar.activation(warm2[:], warm[:], AF.Exp)
    nc.vector.memset(t["ebias"][:], -SHIFT)

    nc.sync.dma_start(t["bq"][:], bq.ap().rearrange("(m p) -> p m", p=P))
    nc.sync.dma_start(t["bk"][:], bk.ap().rearrange("(m p) -> p m", p=P))
    nc.sync.dma_start(t["wq"][:], wq8.ap().rearrange("(ko p) m -> p ko m", p=P))
    nc.sync.dma_start(t["wk"][:], wk8.ap().rearrange("(ko p) m -> p ko m", p=P))
    nc.sync.dma_start(t["wv"][:], wv8.ap().rearrange("(ko p) m -> p ko m", p=P))
    at_r = at8.ap().rearrange("(ko p) n -> p ko n", p=P)
    bt_r = bt8.ap().rearrange("(ko p) n -> p ko n", p=P)
    ct_r = ct8.ap().rearrange("(ko p) n -> p ko n", p=P)
    for ib in range(IB):
        sl = slice(ib * 512, (ib + 1) * 512)
        nc.sync.dma_start(t["at"][:, :, sl], at_r[:, :, sl])
        nc.sync.dma_start(t["bt"][:, :, sl], bt_r[:, :, sl])
        nc.sync.dma_start(t["ct"][:, :, sl], ct_r[:, :, sl])
    nc.sync.dma_start(t["ctr"][:], ctr.ap().rearrange("(m p) n -> p m n", p=P))
    nc.sync.dma_start(t["wo"][:], wo.ap().rearrange("(kt p) d -> p kt d", p=P))
    nc.vector.memset(t["v8"][:], 0.0)
    nc.vector.memset(t["v8"][:, :, :, :, DH:DH + 1], 1.0)
    return pools, t


def _emit_body(nc, tc, mybir, pools, t, outT):
    f32 = mybir.dt.float32
    bf16 = mybir.dt.bfloat16
    fp8 = mybir.dt.float8e4
    u8 = mybir.dt.uint8
    AF = mybir.ActivationFunctionType
    ADD = mybir.AluOpType.add
    MUL = mybir.AluOpType.mult
    DR = mybir.MatmulPerfMode.DoubleRow

    psum, ppool, rin, stage = (pools["psum"], pools["ppool"], pools["rin"],
                               pools["stage"])
    scale = float(DH) ** -0.5

    def ps(nm):
        # every PSUM tile is a 2-bank [P, 2, 512] f32 slot; 4 rotate
        return psum.tile([P, 2, 512], f32, tag="ps", name=nm)

    def proj_blk(which, mb, ib, pq=None, half=0):
        # one q-or-k projection block; optionally share a psum slot
        isl = slice(ib * 512, (ib + 1) * 512)
        dst, w, src, bias = ((t["qt"], t["wq"], t["at"], t["bq"]) if
                             which == "q" else
                             (t["kt"], t["wk"], t["bt"], t["bk"]))
        if pq is None:
            pq = ps(f"pj{which}{mb}{ib}")
        for c in range(KO // 2):
            nc.tensor.matmul(
                pq[:, half, :],
                lhsT=w[:, 2 * c:2 * c + 2, mb * P:(mb + 1) * P],
                rhs=src[:, 2 * c:2 * c + 2, isl],
                start=(c == 0), stop=(c == KO // 2 - 1), perf_mode=DR)
        nc.vector.tensor_scalar_add(
            dst[:, mb, isl], pq[:, half, :], bias[:, mb:mb + 1])
        return pq

    def proj_pair(mb, ib):
        pq = proj_blk("q", mb, ib)
        proj_blk("k", mb, ib, pq=pq, half=1)

    def v_prod2(jp0):
        # V rows for j-pairs jp0, jp0+1 (512 tokens), DR fp8.
        # Quantize-copies split across ACT/DVE to balance the prologue.
        pv = ps(f"pv{jp0}")
        for u in range(2):
            jp = jp0 + u
            for half in range(2):
                csl = slice(jp * 2 * P + half * P, jp * 2 * P + (half + 1) * P)
                for c in range(KO // 2):
                    nc.tensor.matmul(
                        pv[:, u, half * CW:(half + 1) * CW],
                        lhsT=t["ct"][:, 2 * c:2 * c + 2, csl],
                        rhs=t["wv"][:, 2 * c:2 * c + 2, :],
                        start=(c == 0), stop=(c == KO // 2 - 1), perf_mode=DR)
            for half in range(2):
                src = pv[:, u, half * CW:(half + 1) * CW].rearrange(
                    "p (h d) -> p h d", h=4, d=DH)
                dst = t["v8"][:, jp, half, :, 0:DH]
                if jp % 4 == 0:
                    nc.scalar.copy(dst, src)
                else:
                    nc.vector.tensor_copy(out=dst, in_=src)

    def qk_jt(pair, ib, jt, pch, on_dve):
        # scores + exp for one j-tile -> pch [P, 2jp, 2par, 2h, 512]
        isl = slice(ib * 512, ib * 512 + 512)
        jpi, par = divmod(jt % 4, 2)
        st = ps(f"st{jt}")
        for h in range(2):
            nc.tensor.matmul(
                st[:, h, :],
                lhsT=t["kt"][h * DH:(h + 1) * DH, pair, jt * P:(jt + 1) * P],
                rhs=t["qt"][h * DH:(h + 1) * DH, pair, isl],
                start=True, stop=True)
        if on_dve:
            nc.vector.tensor_scalar(
                out=pch[:, jpi, par, :, :].bitcast(u8), in0=st[:],
                scalar1=A8, scalar2=B8, op0=MUL, op1=ADD)
        else:
            nc.scalar.activation(pch[:, jpi, par, :, :], st[:], AF.Exp,
                                 scale=scale, bias=t["ebias"][:])

    def qk_phase(pair, ib, dve_set):
        pchs = []
        for cnk in range(4):
            pch = ppool.tile([P, 2, 2, 2, 512], fp8, tag="pp", name="pp")
            pchs.append(pch)
            for j in range(4):
                jt = 4 * cnk + j
                qk_jt(pair, ib, jt, pch, jt in dve_set)
        return pchs

    def pv_chunk(pair, cnk, pch, po):
        for jpi in range(2):
            jp = 2 * cnk + jpi
            for h in range(2):
                nc.tensor.matmul(
                    po[0:DH + 4, h, :],
                    lhsT=t["v8"][:, jp, :, pair * 2 + h, :],
                    rhs=pch[:, jpi, :, h, :],
                    start=(jp == 0), stop=(jp == KO - 1), perf_mode=DR)

    def norm_resid(pair, ib, po, fast_resid=False):
        isl = slice(ib * 512, ib * 512 + 512)
        # per-head reciprocal of the ones-column rowsum; Q7 broadcasts 1/r
        rs = rin.tile([1, 2, 512], f32, tag="rs", name="rs")
        ri = rin.tile([1, 2, 512], f32, tag="ri", name="ri")
        for h in range(2):
            nc.vector.tensor_copy(out=rs[:, h, :], in_=po[DH:DH + 1, h, :])
            nc.vector.reciprocal_approx_fast(ri[:, h, :], rs[:, h, :])
        for h in range(2):
            rb = rin.tile([DH, 512], f32, tag="rb", name="rb")
            nc.gpsimd.partition_broadcast(rb[:], ri[:, h, :], channels=DH)
            nc.vector.tensor_tensor(
                t["x"][h * DH:(h + 1) * DH, pair, isl],
                po[0:DH, h, :], rb[:], MUL)
        # the drain norm feeds the final outproj with idle engines: run the
        # residual on DVE (2x bf16) instead of Q7 to shorten the chain
        eng = nc.vector if fast_resid else nc.gpsimd
        eng.tensor_tensor(
            t["x"][:, pair, isl], t["x"][:, pair, isl],
            t["ctr"][:, pair, isl], ADD)

    def outproj(ib):
        isl = slice(ib * 512, (ib + 1) * 512)
        for g in range(4):
            pout = ps(f"po3{g}{ib}")
            for u in range(2):
                dc = 2 * g + u
                for kt_ in range(2):
                    nc.tensor.matmul(
                        pout[:, u, :],
                        lhsT=t["wo"][:, kt_, dc * P:(dc + 1) * P],
                        rhs=t["x"][:, kt_, isl],
                        start=(kt_ == 0), stop=(kt_ == 1))
            ot = stage.tile([P, 2, 512], bf16, tag="ot", name="ot")
            nc.any.tensor_copy(out=ot[:], in_=pout[:])
            nc.sync.dma_start(
                outT.ap()[2 * g * P:(2 * g + 2) * P, isl].rearrange(
                    "(u p) n -> p u n", p=P), ot[:])

    def outproj_half(ib, gs, drain=False):
        isl = slice(ib * 512, (ib + 1) * 512)
        for g in gs:
            pout = ps(f"po3{g}{ib}")
            for u in range(2):
                dc = 2 * g + u
                for kt_ in range(2):
                    nc.tensor.matmul(
                        pout[:, u, :],
                        lhsT=t["wo"][:, kt_, dc * P:(dc + 1) * P],
                        rhs=t["x"][:, kt_, isl],
                        start=(kt_ == 0), stop=(kt_ == 1))
            ot = stage.tile([P, 2, 512], bf16, tag="ot", name="ot")
            if drain and g % 2 == 0:
                # both exp engines are idle in the drain: run the psum->sbuf
                # copies on ScalarE/DVE in parallel to halve the tail
                nc.scalar.copy(ot[:], pout[:])
            else:
                nc.any.tensor_copy(out=ot[:], in_=pout[:])
            nc.sync.dma_start(
                outT.ap()[2 * g * P:(2 * g + 2) * P, isl].rearrange(
                    "(u p) n -> p u n", p=P), ot[:])

    # --- emission: chunk-granular exp-paced stream.  The previous combo's
    # PV + norm + outproj are threaded between the current combo's QK
    # chunks so the score/exp stream never starves.
    proj_blk("k", 0, 0)
    proj_blk("q", 0, 0)

    # combo(0,0): JIT kt blocks + V production + own PV per chunk.
    # NOTE: the plan loop re-threads PV(0,0) into combo(0,1); the start=
    # (jp==0) flag makes that re-accumulation recompute the same values, so
    # it is redundant PE work (~1.7us) — but removing the own-PV here
    # (prev po=None) hung the device on HW (NRT_EXEC_UNIT_UNRECOVERABLE)
    # despite simulating fine.  Keep the verified emission.
    pch00 = []
    po00 = None
    for cnk in range(4):
        pch = ppool.tile([P, 2, 2, 2, 512], fp8, tag="pp", name="pp")
        pch00.append(pch)
        for j in range(4):
            jt = 4 * cnk + j
            qk_jt(0, 0, jt, pch, jt in DVE_JT_PROLOGUE)
        if cnk < 3:
            proj_blk("k", 0, cnk + 1)
        v_prod2(2 * cnk)
        if po00 is None:
            po00 = ps("po00")
        pv_chunk(0, cnk, pch, po00)
    proj_blk("q", 0, 1)
    prev = (0, 0, pch00, po00)

    # steady combos; sides[cnk] = PE work emitted after chunk cnk
    plan = [
        (0, 1, {2: [("pj", "q", 0, 2)], 3: [("pj", "q", 1, 0)]}),
        (0, 2, {2: [("pj", "k", 1, 0)], 3: [("pj", "q", 0, 3)]}),
        (0, 3, {1: [("pj", "q", 1, 1)], 2: [("pj", "k", 1, 1)],
                3: [("pj", "q", 1, 2)]}),
        (1, 0, {1: [("pj", "k", 1, 2)], 2: [("pj", "q", 1, 3)],
                3: [("pj", "k", 1, 3)]}),
        (1, 1, {3: [("op", 0, (0, 1))]}),
        (1, 2, {0: [("op", 0, (2, 3))], 3: [("op", 1, (0, 1))]}),
        (1, 3, {0: [("op", 1, (2, 3))], 3: [("op", 2, (0, 1))]}),
    ]
    po_own = None
    for pi, (pair, ib, sides) in enumerate(plan):
        last = pi == len(plan) - 1
        dve_set = (DVE_JT_LAST if last else
                   DVE_JT if pair == 1 else DVE_JT_P0)
        ppair, pib, ppchs, ppo = prev
        pchs = []
        for cnk in range(4):
            # side work first: its DVE/ACT consumer ops land in the engine
            # FIFOs ahead of this chunk's exps, freeing their psum slots fast
            for s in sides.get(cnk, []):
                if s[0] == "pj":
                    proj_blk(s[1], s[2], s[3])
                else:
                    outproj_half(s[1], s[2])
            pch = ppool.tile([P, 2, 2, 2, 512], fp8, tag="pp", name="pp")
            pchs.append(pch)
            for j in range(4):
                jt = 4 * cnk + j
                qk_jt(pair, ib, jt, pch, jt in dve_set)
            # thread previous combo's PV through chunks 0-1, then norm
            if cnk < 2:
                if ppo is None:
                    ppo = ps(f"po{ppair}{pib}")
                pv_chunk(ppair, 2 * cnk, ppchs[2 * cnk], ppo)
                pv_chunk(ppair, 2 * cnk + 1, ppchs[2 * cnk + 1], ppo)
            elif cnk == 2:
                norm_resid(ppair, pib, ppo)
        prev = (pair, ib, pchs, None)
    # drain: last combo's PV + norm, then the remaining outproj blocks
    ppair, pib, ppchs, _ = prev
    ppo = ps(f"po{ppair}{pib}d")
    for cnk in range(4):
        pv_chunk(ppair, cnk, ppchs[cnk], ppo)
    norm_resid(ppair, pib, ppo, fast_resid=True)
    outproj_half(2, (2, 3), drain=True)
    outproj_half(3, (0, 1, 2, 3), drain=True)


def _get_nc(repeat=1):
    key = f"nc{repeat}"
    if key not in _cache:
        _cache[key] = _build(repeat)
    return _cache[key]


def _prep_in_maps(y2_for, y2_back, Wq, bq, Wk, bk, Wv, bv, Wo):
    import ml_dtypes
    bf16 = ml_dtypes.bfloat16
    f8 = ml_dtypes.float8_e4m3

    y2_for = np.asarray(y2_for, dtype=np.float32)
    y2_back = np.asarray(y2_back, dtype=np.float32)
    ct = y2_for + y2_back
    in_maps = []
    for core in range(8):
        b, g = divmod(core, NG)
        c0 = g * CW
        ctr = ct[b, :, c0:c0 + CW].T
        in_maps.append({
            "at8": np.ascontiguousarray(y2_for[b].T).astype(f8),
            "bt8": np.ascontiguousarray(y2_back[b].T).astype(f8),
            "ct8": np.ascontiguousarray(ct[b].T).astype(f8),
            "ctr": np.ascontiguousarray(ctr).astype(bf16),
            "wq8": np.ascontiguousarray(np.asarray(Wq)[:, c0:c0 + CW]).astype(f8),
            "wk8": np.ascontiguousarray(np.asarray(Wk)[:, c0:c0 + CW]).astype(f8),
            "wv8": np.ascontiguousarray(np.asarray(Wv)[:, c0:c0 + CW]).astype(f8),
            "wo": np.ascontiguousarray(np.asarray(Wo)[c0:c0 + CW, :]).astype(bf16),
            "bq": np.ascontiguousarray(np.asarray(bq, dtype=np.float32)[c0:c0 + CW]),
            "bk": np.ascontiguousarray(np.asarray(bk, dtype=np.float32)[c0:c0 + CW]),
        })
    return in_maps


def _combine(results, bv, Wo, bo):
    out = np.zeros((B, N, DIM), dtype=np.float32)
    for core in range(8):
        b = core // NG
        out[b] += results[core]["outT"].astype(np.float32).T
    # v-bias folded out of the device kernel: softmax rows sum to 1, so the
    # +bv on V passes through attention unchanged and contributes bv @ Wo.
    out += (np.asarray(bv, np.float32) @ np.asarray(Wo, np.float32)
            + np.asarray(bo, np.float32))
    return out


def run(y2_for, y2_back, Wq, bq, Wk, bk, Wv, bv, Wo, bo, repeat=1,
        **spmd_kwargs):
    import hashlib
    import os

    from concourse.bass_utils import run_bass_kernel_spmd

    nc = _get_nc(repeat)
    h = hashlib.sha256(nc.to_json_bytes()).hexdigest()[:16]
    os.environ["NEURON_COMPILE_CACHE_URL"] = f"/tmp/neuron-cc-cache-{h}"
    in_maps = _prep_in_maps(y2_for, y2_back, Wq, bq, Wk, bk, Wv, bv, Wo)
    res = run_bass_kernel_spmd(nc, in_maps, core_ids=list(range(8)),
                               **spmd_kwargs)
    return _combine(res.results, bv, Wo, bo), res


def kernel(y2_for, y2_back, Wq, bq, Wk, bk, Wv, bv, Wo, bo):
    out, _ = run(y2_for, y2_back, Wq, bq, Wk, bk, Wv, bv, Wo, bo)
    return out

